# revision 79
# baseline (speedup 1.0000x reference)
"""KagomeConv2D Trainium2 Bass kernel.

Data-parallel over batch (8 cores x 256 batches), fp8 DoubleRow GEMM
with a parity-grid input layout. TimelineSim: 58375 ns, rel err
1.889e-2 (baseline was 65633 ns at 1.345e-2).

  - Host pads x to 10x10, applies the 18 boundary fixups, and splits the
    54 distinct live pixels into three parity classes ((even,even),
    (even,odd), (odd,odd) of (row,col)); each class is stored as a 5x5
    grid x 32-batch stream (75 slots per channel per plane). Every conv
    tap reads an affine 4x4 window of one class grid, so the input
    carries ZERO duplication (a naive slot-window layout ships 114 slots
    for the same 54 pixels).
  - Output positions are row-major (pos = 4i+j). Each conv's 12 live
    positions form 4 contiguous row-runs; each (tap, term) issues 4
    matmuls (N = 64..128) into disjoint PSUM regions of one bank. PE
    cycles are unchanged (cost is N x 0.5 cycles; dead positions never
    computed). start_tensor_calc resets the WHOLE bank, so only the
    first matmul of a bank carries start=True.
  - Matmuls run in fp8e4m3 DoubleRow (0.5 PE cycles/row; the two input-
    channel halves ride the two DoubleRow planes). Accuracy via hi/lo
    split: x ~ x_hi + x_lo, 32*w ~ w_hi + w_lo (all fp8), accumulating
    hi*hi + lo*hi + hi*lo in PSUM f32. TRIM_LO blocks (4 of 8) skip the
    x_lo plane and its term (2/3 PE, half input DMA); each trim adds
    8.8e-5 to the squared rel err. Trims sit at the END of the block
    order: they burn PE faster than their input DMA repays, safe only
    once the stream is ahead.
  - Schedule: per block, [hi*hi x6 groups] then [w-lo x6, oh1 first]
    then [x-lo x6] phases — each block's x_lo plane is needed ~2/3 into
    its PE time. Weights ship as four (hilo, oh) planes (per-slice DMAs
    are HWDGE-generation-bound at ~650ns cadence); the last-needed plane
    (lo, oh0) splits per conv to spread its need across the loop-2
    phases. Taps within a phase are parity-class-sorted to match the
    class-chunk arrival order of block 0. All 8 x tiles stay resident
    (xbufs=8) so the stream never waits on buffer recycling.
  - PE p-state warmup: dependency-free matmuls on a memset tile ramp the
    PE clock (0.65 -> 2.4 GHz over ~3us) during the DMA front.
  - Drain: one activation per (blk, oh, conv) converts the full 16-
    position PSUM (dead cells are bank-reset zeros / stale, discarded on
    host) to bf16 with the 1/32 descale + bias fused, alternating
    ScalarE and DVE. Blocks 0..6 store all three convs per (blk, oh) in
    one DMA, alternating the gpsimd and scalar DGE queues. The last
    block stores per conv on the sync queue, and its final group runs as
    two half-banks whose drains/stores pipeline with the closing
    matmuls (the half1 store rides the gpsimd SWDGE so the two store
    generations run in parallel), leaving only a half-sized drain+store
    chain after the last matmul.
"""

import sys

sys.path.insert(0, "/opt/trn_rl_repo")

import numpy as np

import concourse.bass as bass  # noqa: E402
import concourse.bacc as bacc  # noqa: E402
import concourse.mybir as mybir  # noqa: E402
from concourse.tile import TileContext  # noqa: E402
from concourse.bass_utils import run_bass_kernel_spmd  # noqa: E402

F32 = mybir.dt.float32
BF16 = mybir.dt.bfloat16
F8 = mybir.dt.float8e4
W_SCALE = 32.0  # weights are ~N(0, 1/64) — rescale into fp8e4m3 normal range

B_FULL = 2048
N_CORES = 8
B_CORE = B_FULL // N_CORES
CIN = 256
COUT = 256
BSUB = 32
NBLK = B_CORE // BSUB  # 8
NPOS = 16
NSLOT_TOT = 75  # 3 parity classes x 5x5 grid
# Blocks that skip the x_lo correction plane. Each trim adds ~8.9e-5 to
# the squared rel err (2.66% local on 1/8 of outputs); three fit the
# 2e-2 gate with margin. Trims at the END: they burn PE faster than
# their input DMA repays, safe only once the stream is ahead.
TRIM_LO = {4, 5, 6, 7}

# (dr, dc) taps with mask==1 per conv. conv: 0=up, 1=left, 2=right.
TAPS_UP = [(0, 0), (0, 1), (1, 1), (2, 1), (2, 2)]
TAPS_LEFT = [(1, 1), (2, 0), (2, 1), (2, 2), (3, 1)]
TAPS_RIGHT = [(1, 1), (2, 1), (2, 2), (2, 3), (3, 3)]
CONV_TAPS = [TAPS_UP, TAPS_LEFT, TAPS_RIGHT]

# Parity class of a tap: pixel rows 2i+dr have parity dr%2, cols dc%2.
# Classes: 0=(even,even), 1=(even,odd), 2=(odd,odd).
_CLS = {(0, 0): 0, (0, 1): 1, (1, 1): 2}


def _tap_cls(dr, dc):
    return _CLS[(dr % 2, dc % 2)]


# Grid window origin: pixel (2i+dr, 2j+dc) = class grid [i + dr//2 ...,
# j + dc//2 ...] (odd parities: (2i+1+2k)//2 = i+k with k = (dr-1)//2).
def _tap_origin(dr, dc):
    return ((dr - dr % 2) // 2, (dc - dc % 2) // 2)


# Live (row-major) position runs per conv: (i, j0, length).
# up/right dead {2,3,7,12}; left dead {3,8,12,13} (flat pos = 4i+j).
RUNS = [
    [(0, 0, 2), (1, 0, 3), (2, 0, 4), (3, 1, 3)],  # up
    [(0, 0, 3), (1, 0, 4), (2, 1, 3), (3, 2, 2)],  # left
    [(0, 0, 2), (1, 0, 3), (2, 0, 4), (3, 1, 3)],  # right
]

# Boundary-fixup copies on the zero-padded 10x10 grid (from reference).
DST_R = np.array([0, 0, 0, 0, 1, 2, 3, 4, 6, 7, 8, 9, 9, 9, 8, 6, 4, 2])
DST_C = np.array([0, 1, 2, 3, 5, 6, 7, 8, 9, 9, 9, 9, 7, 5, 3, 1, 0, 0])
SRC_R = np.array([8, 8, 8, 8, 5, 6, 7, 8, 2, 3, 4, 5, 1, 1, 4, 2, 8, 6])
SRC_C = np.array([4, 5, 6, 7, 1, 2, 3, 4, 1, 1, 1, 1, 3, 1, 7, 5, 8, 8])


def build_nc(trim=TRIM_LO, n_warm=5):
    nc = bacc.Bacc(
        "TRN2",
        target_bir_lowering=False,
        debug=False,
        enable_asserts=False,
    )
    # x: [part, blk, hilo, ih, class-grid-slots * b]
    xt = nc.dram_tensor(
        "xt", (128, NBLK, 2, 2, NSLOT_TOT * BSUB), F8, kind="ExternalInput"
    )
    # w: [part, hilo, oh, ci, ti, ih, o] — hilo/oh outermost so each of
    # the four (hilo, oh) planes is one contiguous 3.8KB/partition DMA
    # (per-slice DMAs were HWDGE-generation-bound at 650ns cadence).
    wt = nc.dram_tensor(
        "wt", (128, 2, 2, 3, 5, 2, 128), F8, kind="ExternalInput"
    )
    bias = nc.dram_tensor("bias", (128, 6), F32, kind="ExternalInput")
    outp = nc.dram_tensor(
        "outp", (2, 128, NBLK, 3, NPOS * BSUB), BF16, kind="ExternalOutput"
    )
    xap, wap, bap, oap = xt.ap(), wt.ap(), bias.ap(), outp.ap()

    with TileContext(nc) as tc:
        with (
            tc.tile_pool(name="const", bufs=1) as cpool,
            tc.tile_pool(name="xin", bufs=NBLK) as xpool,
            tc.tile_pool(name="osb", bufs=2) as opool,
            tc.tile_pool(name="ps", bufs=8, space="PSUM") as pspool,
        ):
            w_sb = cpool.tile([128, 2, 2, 3, 5, 2, 128], F8, name="w_sb")
            bias_sb = cpool.tile([128, 6], F32, name="bias_sb")

            # PE p-state warmup (see module docstring).
            warm_sb = cpool.tile([128, 2, 512], F8, name="warm_sb")
            nc.vector.memset(warm_sb[:], 0)
            warm_ps = pspool.tile([128, 512], F32, name="ps", tag="ps")
            for _ in range(n_warm):
                nc.tensor.matmul(
                    warm_ps[:],
                    lhsT=warm_sb[:, :, 0:128],
                    rhs=warm_sb[:],
                    start=True,
                    stop=True,
                    perf_mode=mybir.MatmulPerfMode.DoubleRow,
                )

            x_tiles = [
                xpool.tile(
                    [128, 2, 2, NSLOT_TOT * BSUB], F8, name="x_sb", tag="x_sb"
                )
                for _ in range(NBLK)
            ]

            def _w(hl, oh):
                nc.sync.dma_start(out=w_sb[:, hl, oh], in_=wap[:, hl, oh])

            def _x0cls(hl, cls):
                s0, s1 = cls * 25 * BSUB, (cls + 1) * 25 * BSUB
                nc.sync.dma_start(
                    out=x_tiles[0][:, hl, :, s0:s1],
                    in_=xap[:, 0, hl, :, s0:s1],
                )

            def _xb(blk, split_hi=False):
                if split_hi:
                    # hi plane in two class chunks: with class-sorted taps
                    # the oo class is needed ~0.35us later, easing this
                    # block's arrival pin.
                    s = 2 * 25 * BSUB
                    nc.sync.dma_start(
                        out=x_tiles[blk][:, 0, :, 0:s],
                        in_=xap[:, blk, 0, :, 0:s],
                    )
                    nc.sync.dma_start(
                        out=x_tiles[blk][:, 0, :, s:],
                        in_=xap[:, blk, 0, :, s:],
                    )
                else:
                    nc.sync.dma_start(
                        out=x_tiles[blk][:, 0], in_=xap[:, blk, 0]
                    )
                if blk not in trim:
                    nc.sync.dma_start(
                        out=x_tiles[blk][:, 1], in_=xap[:, blk, 1]
                    )

            # Need-ordered stream matching the three sub-phases of block 0:
            # P1 (hi terms, oh0 then oh1 groups) wants x0-hi + the two hi
            # weight planes; P2 (w-lo terms) the lo planes; P3 (x-lo) the
            # x0-lo classes. Then blocks 1..7 (trims ship hi only).
            _w(0, 0)          # hi weights, oh0
            _x0cls(0, 0)
            _x0cls(0, 1)
            _x0cls(0, 2)
            _w(0, 1)          # hi weights, oh1
            _w(1, 1)          # lo weights, oh1 (w-lo phase runs oh1 first)
            # lo weights oh0 — the last front arrival — split per conv so
            # its need spreads across the three loop-2 oh0 phases.
            for ci in range(3):
                nc.sync.dma_start(
                    out=w_sb[:, 1, 0, ci], in_=wap[:, 1, 0, ci]
                )
            with nc.allow_non_contiguous_dma(reason="tiny one-time bias load"):
                nc.sync.dma_start(out=bias_sb[:], in_=bap)
            if 0 not in trim:
                _x0cls(1, 0)
                _x0cls(1, 1)
                _x0cls(1, 2)
            for blk in range(1, NBLK):
                _xb(blk, split_hi=(blk == 1))

            def _mm_phase(x_sb, ps, oh, ci, wsel, xsel, first, last, rsl=None):
                # start_tensor_calc resets the WHOLE PSUM bank, so only the
                # very first matmul of a bank may carry start=True; all
                # later run-regions accumulate onto the bank-wide zeros.
                # Taps are ordered by parity class so the last class chunk
                # of a block's x DMA is needed as late as possible.
                runs = RUNS[ci] if rsl is None else RUNS[ci][rsl]
                taps = list(enumerate(CONV_TAPS[ci]))
                taps.sort(key=lambda t: _tap_cls(*t[1]))
                last_ti = taps[-1][0]
                for ti, (dr, dc) in taps:
                    cls = _tap_cls(dr, dc)
                    r0, c0 = _tap_origin(dr, dc)
                    for ri, (i, j0, ln) in enumerate(runs):
                        src = (cls * 25 + (r0 + i) * 5 + (c0 + j0)) * BSUB
                        dst = (4 * i + j0) * BSUB
                        nc.tensor.matmul(
                            ps[:, dst : dst + ln * BSUB],
                            lhsT=w_sb[:, wsel, oh, ci, ti],
                            rhs=x_sb[:, xsel, :, src : src + ln * BSUB],
                            start=first and ti == taps[0][0] and ri == 0,
                            stop=last and ti == last_ti
                            and ri == len(runs) - 1,
                            perf_mode=mybir.MatmulPerfMode.DoubleRow,
                            skip_group_check=True,
                        )

            for blk in range(NBLK):
                x_sb = x_tiles[blk]
                is_trim = blk in trim
                # Phase order: [hi*hi + w-lo] for all six (oh, conv) psum
                # groups first, the x-lo correction terms last — the x_lo
                # plane of this block is only needed ~2/3 into the block's
                # PE time, giving the DMA stream slack. Six PSUM banks stay
                # open concurrently (pool has 8).
                pss = {}
                osbs = {}
                for oh in range(2):
                    osbs[oh] = opool.tile(
                        [128, 3, NPOS * BSUB], BF16, name=f"osb{oh}",
                        tag=f"osb{oh}",
                    )
                    for ci in range(3):
                        pss[(oh, ci)] = pspool.tile(
                            [128, 512], F32, name="ps", tag="ps"
                        )
                if blk == NBLK - 1:
                    # Last block: per-group phases (groups close ~0.8us
                    # apart); drains alternate between ScalarE and the
                    # otherwise-idle DVE so they pipeline with the closes,
                    # and each conv stores separately on the low-latency
                    # sync queue. The final group drains in two position-
                    # halves so only a half-sized drain+store chain trails
                    # the last matmul.
                    terms_l = [(0, 0, True, False), (1, 0, False, is_trim)]
                    if not is_trim:
                        terms_l.append((0, 1, False, True))
                    for oh in range(2):
                        for ci in range(3):
                            ps = pss[(oh, ci)]
                            k = ci * 2 + oh
                            if oh == 1 and ci == 2:
                                # Final group: compute + drain + store in two
                                # position halves on SEPARATE psum banks
                                # (bank-granular dep tracking would other-
                                # wise serialize half2's writes behind
                                # half1's drain) — the first half's drain
                                # and store-DGE generation overlap the second
                                # half's matmuls, so only a half-sized chain
                                # trails the very last matmul.
                                ps2 = pspool.tile(
                                    [128, 512], F32, name="ps", tag="ps"
                                )
                                splits = (
                                    (slice(0, 2), 0, 256),
                                    (slice(2, 4), 256, 256),
                                )
                                for hx, (rsl, off, ln) in enumerate(splits):
                                    psh = (ps, ps2)[hx]
                                    for (ws, xs, fi, la) in terms_l:
                                        _mm_phase(
                                            x_sb, psh, oh, ci, ws, xs,
                                            fi, la, rsl,
                                        )
                                    osl = osbs[oh][:, ci, off : off + ln]
                                    psl = psh[:, off : off + ln]
                                    if hx == 0:
                                        nc.scalar.activation(
                                            out=osl,
                                            in_=psl,
                                            func=mybir.ActivationFunctionType.Identity,
                                            scale=1.0 / W_SCALE,
                                            bias=bias_sb[:, k : k + 1],
                                        )
                                    else:
                                        nc.vector.tensor_scalar(
                                            out=osl,
                                            in0=psl,
                                            scalar1=1.0 / W_SCALE,
                                            scalar2=bias_sb[:, k : k + 1],
                                            op0=mybir.AluOpType.mult,
                                            op1=mybir.AluOpType.add,
                                        )
                                    # half1 rides the SWDGE (gpsimd) queue:
                                    # its descriptor gen runs parallel to
                                    # the HWDGE gens of the other stores.
                                    eng = nc.gpsimd if hx == 0 else nc.sync
                                    eng.dma_start(
                                        out=oap[oh, :, blk, ci, off : off + ln],
                                        in_=osl,
                                    )
                                continue
                            for (ws, xs, fi, la) in terms_l:
                                _mm_phase(x_sb, ps, oh, ci, ws, xs, fi, la)
                            if (oh * 3 + ci) % 2 == 1:
                                nc.scalar.activation(
                                    out=osbs[oh][:, ci],
                                    in_=ps[:],
                                    func=mybir.ActivationFunctionType.Identity,
                                    scale=1.0 / W_SCALE,
                                    bias=bias_sb[:, k : k + 1],
                                )
                            else:
                                nc.vector.tensor_scalar(
                                    out=osbs[oh][:, ci],
                                    in0=ps[:],
                                    scalar1=1.0 / W_SCALE,
                                    scalar2=bias_sb[:, k : k + 1],
                                    op0=mybir.AluOpType.mult,
                                    op1=mybir.AluOpType.add,
                                )
                            nc.sync.dma_start(
                                out=oap[oh, :, blk, ci], in_=osbs[oh][:, ci]
                            )
                else:
                    for oh in range(2):
                        for ci in range(3):
                            _mm_phase(
                                x_sb, pss[(oh, ci)], oh, ci, 0, 0, True, False
                            )
                    # w-lo phase runs oh1 before oh0: the oh1 lo weights
                    # are the last front arrival, and this order minimizes
                    # (arrival + remaining-PE) over the front items.
                    for oh in (1, 0):
                        for ci in range(3):
                            _mm_phase(
                                x_sb, pss[(oh, ci)], oh, ci, 1, 0, False,
                                is_trim,
                            )

                if blk == NBLK - 1:
                    continue
                for oh in range(2):
                    for ci in range(3):
                        ps = pss[(oh, ci)]
                        if not is_trim:
                            _mm_phase(x_sb, ps, oh, ci, 0, 1, False, True)
                        k = ci * 2 + oh
                        if (oh * 3 + ci) % 2 == 0:
                            nc.scalar.activation(
                                out=osbs[oh][:, ci],
                                in_=ps[:],
                                func=mybir.ActivationFunctionType.Identity,
                                scale=1.0 / W_SCALE,
                                bias=bias_sb[:, k : k + 1],
                            )
                        else:
                            nc.vector.tensor_scalar(
                                out=osbs[oh][:, ci],
                                in0=ps[:],
                                scalar1=1.0 / W_SCALE,
                                scalar2=bias_sb[:, k : k + 1],
                                op0=mybir.AluOpType.mult,
                                op1=mybir.AluOpType.add,
                            )
                    if (blk * 2 + oh) % 2 == 0:
                        nc.gpsimd.dma_start(
                            out=oap[oh, :, blk], in_=osbs[oh][:]
                        )
                    else:
                        nc.scalar.dma_start(
                            out=oap[oh, :, blk], in_=osbs[oh][:]
                        )

    nc.compile()
    return nc


def _grid_gather_indices():
    """rows/cols into the padded 10x10 image for the 75-slot stream."""
    rows = np.zeros((NSLOT_TOT,), np.intp)
    cols = np.zeros((NSLOT_TOT,), np.intp)
    for cls, (pr, pc) in enumerate([(0, 0), (0, 1), (1, 1)]):
        for R in range(5):
            for C in range(5):
                rows[cls * 25 + R * 5 + C] = 2 * R + pr
                cols[cls * 25 + R * 5 + C] = 2 * C + pc
    return rows, cols


def prep_inputs(x, w_up, b_up, w_left, b_left, w_right, b_right):
    import ml_dtypes

    x = np.asarray(x, dtype=np.float32)
    ws = [np.asarray(w_up), np.asarray(w_left), np.asarray(w_right)]
    bs = [np.asarray(b_up), np.asarray(b_left), np.asarray(b_right)]

    # Padded image + boundary fixups (host side).
    xpad = np.zeros((B_FULL, CIN, 10, 10), np.float32)
    xpad[:, :, 1:9, 1:9] = x
    xpad[:, :, DST_R, DST_C] = xpad[:, :, SRC_R, SRC_C]

    rows, cols = _grid_gather_indices()
    g = xpad[:, :, rows, cols]  # [B, C, 75] f32
    f8 = ml_dtypes.float8_e4m3fn
    x_hi = g.astype(f8)
    x_lo = (g - x_hi.astype(np.float32)).astype(f8)
    # [2hilo, B, C, 75] -> [128p, core, blk, hilo, ih, slot, b]
    gq = np.stack([x_hi, x_lo])
    gq = gq.reshape(2, N_CORES, NBLK, BSUB, 2, 128, NSLOT_TOT)
    gq = np.ascontiguousarray(gq.transpose(5, 1, 2, 0, 4, 6, 3))
    g = gq.reshape(128, N_CORES, NBLK, 2, 2, NSLOT_TOT * BSUB)

    # Weights: wt32[p, conv, oh, ti, ih, o] = W_SCALE * w[conv][oh*128+o,
    # ih*128+p, dr, dc], then split into fp8 hi + lo residual planes.
    wt32 = np.empty((128, 3, 2, 5, 2, 128), np.float32)
    for ci, taps in enumerate(CONV_TAPS):
        for ti, (dr, dc) in enumerate(taps):
            w = ws[ci][:, :, dr, dc]  # [O, I]
            w4 = w.reshape(2, 128, 2, 128).transpose(3, 0, 2, 1)
            wt32[:, ci, :, ti, :, :] = w4 * W_SCALE
    w_hi = wt32.astype(f8)
    w_lo = (wt32 - w_hi.astype(np.float32)).astype(f8)
    # wt32 is [p, ci, oh, ti, ih, o]; stack hilo then reorder to
    # [p, hilo, oh, ci, ti, ih, o].
    wt = np.stack([w_hi, w_lo], axis=1)  # [p, hilo, ci, oh, ti, ih, o]
    wt = np.ascontiguousarray(wt.transpose(0, 1, 3, 2, 4, 5, 6))

    bias = np.empty((128, 6), np.float32)
    for ci, b in enumerate(bs):
        bias[:, ci * 2 + 0] = b[:128]
        bias[:, ci * 2 + 1] = b[128:]
    return g, wt, np.ascontiguousarray(bias)


# Live row-major positions and their flat cells on the 8x8 output.
def _conv_live():
    live = []
    for ci in range(3):
        pos_list, cell_list = [], []
        for (i, j0, ln) in RUNS[ci]:
            for j in range(j0, j0 + ln):
                pos_list.append(4 * i + j)
                if ci == 0:
                    cell_list.append(16 * i + 2 * j)
                elif ci == 1:
                    cell_list.append(16 * i + 8 + 2 * j)
                else:
                    cell_list.append(16 * i + 8 + 2 * j + 1)
        live.append((np.array(pos_list), np.array(cell_list)))
    return live


CONV_LIVE = _conv_live()

_NC_CACHE = {}


def _get_nc():
    if "nc" not in _NC_CACHE:
        _NC_CACHE["nc"] = build_nc()
    return _NC_CACHE["nc"]


def run(inputs, trace=False):
    g, wt, bias = prep_inputs(
        inputs["x"], inputs["w_up"], inputs["b_up"], inputs["w_left"],
        inputs["b_left"], inputs["w_right"], inputs["b_right"],
    )
    nc = _get_nc()
    in_maps = [
        {"xt": np.ascontiguousarray(g[:, i]), "wt": wt, "bias": bias}
        for i in range(N_CORES)
    ]
    res = run_bass_kernel_spmd(nc, in_maps, core_ids=list(range(N_CORES)), trace=trace)

    out = np.zeros((B_FULL, COUT, 64), np.float32)
    # packed: [core, 2oh, 128p, blk, 3conv, 16pos, 32b] bf16
    packed = np.stack([r["outp"] for r in res.results]).astype(np.float32)
    packed = packed.reshape(N_CORES, 2, 128, NBLK, 3, NPOS, BSUB)
    # -> [core, blk, b, oh, p, conv, pos] -> [B, O, conv, pos]
    full = packed.transpose(0, 3, 6, 1, 2, 4, 5).reshape(
        B_FULL, COUT, 3, NPOS
    )
    for ci in range(3):
        pos, cells = CONV_LIVE[ci]
        out[:, :, cells] = full[:, :, ci, pos]
    return out.reshape(B_FULL, COUT, 8, 8), res


def kernel(**inputs):
    out, _ = run(inputs)
    return out
